# revision 28
# baseline (speedup 1.0000x reference)
"""Trainium2 Bass kernel: LayerNorm + QKV projection + RoPE (dense transformer).

Full inputs in, full outputs out. Internally shards the 8192 token rows
(b=2 x n=4096) across 8 NeuronCores (data parallel, 1024 tokens/core).

Per-core pipeline (v6 — PE is a pure fp16 matmul stream):
  Phase A: DMA x tile [128, 2048]; LayerNorm via bn_stats/bn_aggr (VectorE)
     writing normalized fp16; one DMA-xbar transpose per m-tile lands it in
     xnt [d, k, t]. ln_gamma is folded into the weights host-side (and
     ln_beta, when nonzero, becomes a per-column bias added post-matmul), so
     no PE transposes or ScalarE fixup copies exist at all. xnt is
     double-buffered; the next rep's phase A is emitted between this rep's
     phase-B chunks (software pipeline), so the PE never idles.
  Phase B: QKV matmuls in fp16, 512-col moving operands accumulated over 16
     k-tiles in PSUM (8-bank rotation); weight half-chunks stream via one
     coarse DMA each (32KB/partition contiguous), prefetched 2 chunks deep;
     ScalarE drains PSUM->SBUF casting fp16; RoPE on VectorE (q) and
     VectorE+GpSimd (k); fp16 output blocks DMA out (v from the ACT queue).

Pools and constants are created once; body repetitions (used by the timing
harness) share them, so reps pipeline back-to-back without drain barriers.
"""

import os
from contextlib import ExitStack

import numpy as np

import concourse.bass as bass
import concourse.tile as tile
from concourse import bacc, mybir
from concourse.bass_utils import run_bass_kernel_spmd

# Problem shapes (hardcoded per contract)
B, N, DM = 2, 4096, 2048
NCORES = 8
TOK = B * N            # 8192 total token rows
TPC = TOK // NCORES    # 1024 tokens per core
P = 128
MT = TPC // P          # 8 m-tiles per core
KT = DM // P           # 16 k-tiles (contraction)
HEADS, HD = 16, 128
ECW = 1024             # weight-chunk width (half the e range)
NCH = DM // ECW        # 2 chunks
NB = ECW // 512        # matmul n-slices per chunk (PSUM bank = 512 fp32)
HPC = ECW // HD        # heads per chunk = 8
LN_EPS = 1e-5
ROPE_BASE = 10000.0

F32 = mybir.dt.float32
# Matmul input dtype: float16 (default; ~tf32 accuracy) or bfloat16.
MM_DT = getattr(mybir.dt, os.environ.get("QKV_MM_DT", "float16"))
# Staging/output dtype: fp16 halves output DMA traffic and doubles RoPE
# vector throughput; quantization (~5e-4 rel) is far inside the 2e-2 gate.
OUT_DT = getattr(mybir.dt, os.environ.get("QKV_OUT_DT", "float16"))

_CACHE = {}


def _build_nc(body_reps=None, with_bias=False):
    if body_reps is None:
        body_reps = int(os.environ.get("QKV_BODY_REPS", "1"))
    nc = bacc.Bacc("TRN2", target_bir_lowering=False, debug=False,
                   enable_asserts=False, num_devices=NCORES)

    x = nc.dram_tensor("x", [TPC, DM], F32, kind="ExternalInput").ap()
    wts = [
        nc.dram_tensor(f"w{n}", [NCH, P, KT * ECW], MM_DT,
                       kind="ExternalInput").ap()
        for n in "qkv"
    ]
    # ln_beta folded through the projections: per-output-column bias,
    # only materialized when ln_beta is nonzero.
    biases = None
    if with_bias:
        biases = [
            nc.dram_tensor(f"b{n}", [1, DM], OUT_DT,
                           kind="ExternalInput").ap()
            for n in "qkv"
        ]
    cosT = nc.dram_tensor("cosT", [P, MT, HD // 2], F32, kind="ExternalInput").ap()
    sinT = nc.dram_tensor("sinT", [P, MT, HD // 2], F32, kind="ExternalInput").ap()
    outs = [
        nc.dram_tensor(f"{n}_out", [TPC, DM], OUT_DT,
                       kind="ExternalOutput").ap()
        for n in "qkv"
    ]

    with tile.TileContext(nc) as tc:
        with ExitStack() as ctx:
            st = _make_state(ctx, tc, cosT, sinT, biases)
            # Software pipeline across body reps: phase A of rep r+1 is
            # emitted tile-by-tile between phase B chunks of rep r, so the
            # PE never sees a long transpose-only (non-matmul) stretch.
            xnt_cur = st["xnt"].tile([P, KT, TPC], MM_DT, name="xnt")
            for m in range(MT):
                _phase_a_tile(tc, st, x, xnt_cur, m)
            for rep in range(body_reps):
                if rep + 1 < body_reps:
                    xnt_next = st["xnt"].tile([P, KT, TPC], MM_DT, name="xnt")

                    def interleave(ci, _xn=xnt_next):
                        for m in _A_SCHED.get(ci, ()):
                            _phase_a_tile(tc, st, x, _xn, m)
                else:
                    xnt_next = None

                    def interleave(ci):
                        pass
                _phase_b(tc, st, wts, outs, xnt_cur, interleave)
                xnt_cur = xnt_next
    nc.compile()
    return nc


# phase-A m-tiles of the NEXT rep emitted after these phase-B chunk indices
_A_SCHED = {0: (0, 1), 1: (2, 3), 2: (4, 5), 3: (6, 7)}


def _make_state(ctx, tc, cosT, sinT, biases):
    """Pools + one-time constants, shared by all body reps."""
    nc = tc.nc
    st = {}
    singles = ctx.enter_context(tc.tile_pool(name="singles", bufs=1))
    st["xpool"] = ctx.enter_context(tc.tile_pool(name="xpool", bufs=3))
    st["xh"] = ctx.enter_context(tc.tile_pool(name="xh", bufs=3))
    st["stats"] = ctx.enter_context(tc.tile_pool(name="stats", bufs=4))
    st["xnt"] = ctx.enter_context(tc.tile_pool(name="xnt", bufs=2))
    st["wt"] = ctx.enter_context(tc.tile_pool(name="wt", bufs=2))
    st["stage"] = ctx.enter_context(tc.tile_pool(name="stage", bufs=4))
    st["rope"] = ctx.enter_context(tc.tile_pool(name="rope", bufs=3))
    # All 8 PSUM banks rotate through matmul accumulators
    st["psumB"] = ctx.enter_context(
        tc.tile_pool(name="psumB", bufs=8, space="PSUM"))

    eps_t = singles.tile([P, 1], F32)
    nc.vector.memset(eps_t, LN_EPS)
    st["eps"] = eps_t
    cos_sb = singles.tile([P, MT, HD // 2], F32)
    nc.sync.dma_start(out=cos_sb, in_=cosT)
    st["cos"] = cos_sb
    sin_sb = singles.tile([P, MT, HD // 2], F32)
    nc.sync.dma_start(out=sin_sb, in_=sinT)
    st["sin"] = sin_sb
    st["bias"] = None
    if biases is not None:
        st["bias"] = []
        for bi, b_dram in enumerate(biases):
            b_sb = singles.tile([P, DM], OUT_DT, name=f"bias{bi}")
            # broadcast the [1, DM] row into all 128 partitions
            b_bc = bass.AP(tensor=b_dram.tensor, offset=b_dram.offset,
                           ap=[[0, P], b_dram.ap[1]])
            nc.sync.dma_start(out=b_sb, in_=b_bc)
            st["bias"].append(b_sb)
    return st


def _phase_a_tile(tc, st, x, xnt, m):
    """LayerNorm of one 128-token m-tile, then DMA-xbar transpose into xnt."""
    nc = tc.nc
    x_t = st["xpool"].tile([P, DM], F32)
    nc.sync.dma_start(out=x_t, in_=x[m * P:(m + 1) * P, :])

    xg = x_t.rearrange("p (g s) -> p g s", s=512)
    stt = st["stats"].tile([P, 4, nc.vector.BN_STATS_DIM], F32)
    for g in range(4):
        nc.vector.bn_stats(out=stt[:, g, :], in_=xg[:, g, :])
    mv = st["stats"].tile([P, nc.vector.BN_AGGR_DIM], F32)
    nc.vector.bn_aggr(out=mv, in_=stt)

    # rsig = 1/sqrt(var + eps)
    rsig = st["stats"].tile([P, 1], F32)
    nc.scalar.activation(out=rsig, in_=mv[:, 1:2],
                         func=mybir.ActivationFunctionType.Sqrt,
                         bias=st["eps"], scale=1.0)
    nc.vector.reciprocal(out=rsig, in_=rsig)

    # xh = (x - mu) * rsig, cast to the matmul dtype
    xh = st["xh"].tile([P, DM], MM_DT)
    nc.vector.tensor_scalar(out=xh, in0=x_t,
                            scalar1=mv[:, 0:1], scalar2=rsig,
                            op0=mybir.AluOpType.subtract,
                            op1=mybir.AluOpType.mult)

    # One xbar transpose per m-tile: xnt[p, k, t] = xh[t, k*128+p]
    nc.scalar.dma_start_transpose(out=xnt[:, :, m * P:(m + 1) * P], in_=xh)


def _phase_b(tc, st, wts, outs, xnt, interleave):
    nc = tc.nc
    cos_sb, sin_sb = st["cos"], st["sin"]

    # ---- Phase B: QKV matmuls + RoPE + store ----
    # Weight chunks are prefetched at depth 2: chunk i+2's DMA is emitted
    # right after chunk i's matmuls, giving the transfer a full chunk of
    # compute (~50us) to land before it is consumed.
    chunks = [(wi, c) for wi in range(3) for c in range(NCH)]

    def fetch_w(i):
        wi, c = chunks[i]
        w_sb = st["wt"].tile([P, KT * ECW], MM_DT)
        nc.sync.dma_start(out=w_sb, in_=wts[wi][c])
        return w_sb

    wbufs = {0: fetch_w(0), 1: fetch_w(1)}

    for ci, (wi, c) in enumerate(chunks):
        w_sb = wbufs.pop(ci)
        o_dram = outs[wi]
        for m in range(MT):
            accs = [st["psumB"].tile([P, 512], F32, space="PSUM",
                                     name="psB")
                    for _ in range(NB)]
            for n in range(NB):
                for k in range(KT):
                    off = k * ECW + n * 512
                    nc.tensor.matmul(
                        accs[n], lhsT=xnt[:, k, m * P:(m + 1) * P],
                        rhs=w_sb[:, off:off + 512],
                        start=(k == 0), stop=(k == KT - 1),
                    )

            stg = st["stage"].tile([P, ECW], OUT_DT)
            for n in range(NB):
                nc.scalar.activation(
                    out=stg[:, n * 512:(n + 1) * 512], in_=accs[n],
                    func=mybir.ActivationFunctionType.Copy)

            if st["bias"] is not None:  # folded ln_beta (zero for this net)
                nc.vector.tensor_add(
                    stg, stg, st["bias"][wi][:, c * ECW:(c + 1) * ECW])

            if wi < 2:  # rope on q and k
                # q: all 8 heads on VectorE. k: heads split DVE/GpSimd so
                # the per-group RoPE latency stays well under the matmul
                # group time (GpSimd is ~1.9x slower per element).
                if wi == 0:
                    splits = [(nc.vector, 0, HPC)]
                else:
                    splits = [(nc.vector, 0, HPC // 2),
                              (nc.gpsimd, HPC // 2, HPC)]
                ov = stg.rearrange("p (h d) -> p h d", d=HD)
                cos_m = cos_sb[:, m, :]
                sin_m = sin_sb[:, m, :]
                for eng, h0, h1 in splits:
                    nh = h1 - h0
                    q1 = ov[:, h0:h1, 0:HD // 2]
                    q2 = ov[:, h0:h1, HD // 2:HD]
                    cos_b = bass.AP(tensor=cos_m.tensor, offset=cos_m.offset,
                                    ap=[cos_m.ap[0], [0, nh], cos_m.ap[1]])
                    sin_b = bass.AP(tensor=sin_m.tensor, offset=sin_m.offset,
                                    ap=[sin_m.ap[0], [0, nh], sin_m.ap[1]])
                    ta = st["rope"].tile([P, nh, HD // 2], OUT_DT,
                                         name=f"ropeA{wi}{h0}")
                    tb = st["rope"].tile([P, nh, HD // 2], OUT_DT,
                                         name=f"ropeB{wi}{h0}")
                    eng.tensor_mul(ta, q1, sin_b)      # A = q1*sin
                    eng.tensor_mul(tb, q2, sin_b)      # B = q2*sin
                    eng.tensor_mul(q1, q1, cos_b)      # q1 = q1*cos
                    eng.tensor_sub(q1, q1, tb)         # q1 -= B
                    eng.tensor_mul(q2, q2, cos_b)      # q2 = q2*cos
                    eng.tensor_add(q2, q2, ta)         # q2 += A

            # v (no RoPE) stores from the ACT queue: zero-wait issue right
            # after ACT's own PSUM->SBUF copy, and it keeps RoPE-gated
            # stores off the SP queue ahead of weight prefetches.
            dma_eng = nc.scalar if wi == 2 else nc.sync
            dma_eng.dma_start(
                out=o_dram[m * P:(m + 1) * P, c * ECW:(c + 1) * ECW],
                in_=stg)

        if ci + 2 < len(chunks):
            wbufs[ci + 2] = fetch_w(ci + 2)
        interleave(ci)


def _host_prep(x, ln_gamma, ln_beta, wq, wk, wv):
    """Shard/layout inputs. Returns per-core input maps.

    ln_gamma is folded into the projection weights (q = xn_hat @ (gamma*W).T);
    ln_beta folds to a per-output-column bias beta @ W.T, included in the
    maps only when nonzero (the kernel program is built accordingly).
    """
    xf = np.ascontiguousarray(x.reshape(TOK, DM), dtype=np.float32)
    wdt = mybir.dt.np(MM_DT)
    gamma = np.asarray(ln_gamma, np.float32)
    beta = np.asarray(ln_beta, np.float32)

    def tile_w(w):
        wt = np.asarray(w, np.float32).T * gamma[:, None]  # [d, e]
        # [NCH, P, KT*ECW]: [c, p, k*ECW+j] = wt[k*128+p, c*ECW+j]
        t = wt.reshape(KT, P, NCH, ECW).transpose(2, 1, 0, 3)
        return np.ascontiguousarray(t).reshape(NCH, P, KT * ECW).astype(wdt)

    wq_t, wk_t, wv_t = tile_w(wq), tile_w(wk), tile_w(wv)
    with_bias = bool(np.any(beta))
    if with_bias:
        odt = mybir.dt.np(OUT_DT)
        b_maps = {
            f"b{n}": np.ascontiguousarray(
                (beta @ np.asarray(w, np.float32).T).reshape(1, DM)
                .astype(odt))
            for n, w in (("q", wq), ("k", wk), ("v", wv))
        }
    else:
        b_maps = {}

    # Build RoPE tables with jax.numpy, matching the reference's fp32 trig
    # bit-for-bit (numpy's fp32 cos differs by ~3e-4 at large arguments).
    import jax.numpy as jnp
    inv_freq = 1.0 / (ROPE_BASE ** (jnp.arange(0, HD, 2, dtype=jnp.float32) / HD))
    t = jnp.arange(N, dtype=jnp.float32)
    freqs = jnp.einsum("i,j->ij", t, inv_freq)  # [N, 64]
    cos_full = np.asarray(jnp.cos(freqs), dtype=np.float32)
    sin_full = np.asarray(jnp.sin(freqs), dtype=np.float32)

    in_maps = []
    for c in range(NCORES):
        pos0 = (c * TPC) % N
        cos_c = np.ascontiguousarray(
            cos_full[pos0:pos0 + TPC].reshape(MT, P, HD // 2).transpose(1, 0, 2))
        sin_c = np.ascontiguousarray(
            sin_full[pos0:pos0 + TPC].reshape(MT, P, HD // 2).transpose(1, 0, 2))
        in_maps.append({
            "x": np.ascontiguousarray(xf[c * TPC:(c + 1) * TPC]),
            "wq": wq_t, "wk": wk_t, "wv": wv_t,
            "cosT": cos_c, "sinT": sin_c,
            **b_maps,
        })
    return in_maps


def _assemble(res_list, name):
    full = np.concatenate([res_list[c][name] for c in range(NCORES)], axis=0)
    return np.ascontiguousarray(
        full.reshape(B, N, HEADS, HD).transpose(0, 2, 1, 3)
        .astype(np.float32))


def kernel(x, ln_gamma, ln_beta, wq, wk, wv, num_heads, _trace=False):
    assert int(num_heads) == HEADS
    in_maps = _host_prep(x, ln_gamma, ln_beta, wq, wk, wv)
    with_bias = "bq" in in_maps[0]
    key = f"nc_bias{with_bias}"
    if key not in _CACHE:
        _CACHE[key] = _build_nc(with_bias=with_bias)
    nc = _CACHE[key]
    r = run_bass_kernel_spmd(nc, in_maps, core_ids=list(range(NCORES)),
                             trace=_trace)
    if _trace:
        _CACHE["last_results"] = r
    q = _assemble(r.results, "q_out")
    k = _assemble(r.results, "k_out")
    v = _assemble(r.results, "v_out")
    return q, k, v


# revision 30
# speedup vs baseline: 1.4025x; 1.4025x over previous
"""Trainium2 Bass kernel: LayerNorm + QKV projection + RoPE (dense transformer).

Full inputs in, full outputs out. Internally shards the 8192 token rows
(b=2 x n=4096) across 8 NeuronCores (data parallel, 1024 tokens/core).

Per-core pipeline (v6 — PE is a pure fp16 matmul stream):
  Phase A: DMA x tile [128, 2048]; LayerNorm via bn_stats/bn_aggr (VectorE)
     writing normalized fp16; one DMA-xbar transpose per m-tile lands it in
     xnt [d, k, t]. ln_gamma is folded into the weights host-side (and
     ln_beta, when nonzero, becomes a per-column bias added post-matmul), so
     no PE transposes or ScalarE fixup copies exist at all. xnt is
     double-buffered; the next rep's phase A is emitted between this rep's
     phase-B chunks (software pipeline), so the PE never idles.
  Phase B: QKV matmuls in fp16, 512-col moving operands accumulated over 16
     k-tiles in PSUM (8-bank rotation); weight half-chunks stream via one
     coarse DMA each (32KB/partition contiguous), prefetched 2 chunks deep;
     ScalarE drains PSUM->SBUF casting fp16; RoPE on VectorE (q) and
     VectorE+GpSimd (k); fp16 output blocks DMA out (v from the ACT queue).

Pools and constants are created once; body repetitions (used by the timing
harness) share them, so reps pipeline back-to-back without drain barriers.
"""

import os
from contextlib import ExitStack

import numpy as np

import concourse.bass as bass
import concourse.tile as tile
from concourse import bacc, mybir
from concourse.bass_utils import run_bass_kernel_spmd

# Problem shapes (hardcoded per contract)
B, N, DM = 2, 4096, 2048
NCORES = 8
TOK = B * N            # 8192 total token rows
TPC = TOK // NCORES    # 1024 tokens per core
P = 128
MT = TPC // P          # 8 m-tiles per core
KT = DM // P           # 16 k-tiles (contraction)
HEADS, HD = 16, 128
ECW = 1024             # weight-chunk width (half the e range)
NCH = DM // ECW        # 2 chunks
NB = ECW // 512        # matmul n-slices per chunk (PSUM bank = 512 fp32)
HPC = ECW // HD        # heads per chunk = 8
LN_EPS = 1e-5
ROPE_BASE = 10000.0

F32 = mybir.dt.float32
# Matmul input dtype: float16 (default; ~tf32 accuracy) or bfloat16.
MM_DT = getattr(mybir.dt, os.environ.get("QKV_MM_DT", "float16"))
# Staging/output dtype: fp16 halves output DMA traffic and doubles RoPE
# vector throughput; quantization (~5e-4 rel) is far inside the 2e-2 gate.
OUT_DT = getattr(mybir.dt, os.environ.get("QKV_OUT_DT", "float16"))

_CACHE = {}


def _build_nc(body_reps=None, with_bias=False):
    if body_reps is None:
        body_reps = int(os.environ.get("QKV_BODY_REPS", "1"))
    nc = bacc.Bacc("TRN2", target_bir_lowering=False, debug=False,
                   enable_asserts=False, num_devices=NCORES)

    x = nc.dram_tensor("x", [TPC, DM], F32, kind="ExternalInput").ap()
    wts = [
        nc.dram_tensor(f"w{n}", [NCH, P, KT * ECW], MM_DT,
                       kind="ExternalInput").ap()
        for n in "qkv"
    ]
    # ln_beta folded through the projections: per-output-column bias,
    # only materialized when ln_beta is nonzero.
    biases = None
    if with_bias:
        biases = [
            nc.dram_tensor(f"b{n}", [1, DM], OUT_DT,
                           kind="ExternalInput").ap()
            for n in "qkv"
        ]
    cosT = nc.dram_tensor("cosT", [P, MT, HD // 2], F32, kind="ExternalInput").ap()
    sinT = nc.dram_tensor("sinT", [P, MT, HD // 2], F32, kind="ExternalInput").ap()
    outs = [
        nc.dram_tensor(f"{n}_out", [TPC, DM], OUT_DT,
                       kind="ExternalOutput").ap()
        for n in "qkv"
    ]

    with tile.TileContext(nc) as tc:
        with ExitStack() as ctx:
            st = _make_state(ctx, tc, cosT, sinT, biases)
            # Software pipeline across body reps: phase A of rep r+1 is
            # emitted tile-by-tile between phase B chunks of rep r, so the
            # PE never sees a long transpose-only (non-matmul) stretch.
            xnt_cur = st["xnt"].tile([P, KT, TPC], MM_DT, name="xnt")
            for m in range(MT):
                _phase_a_tile(tc, st, x, xnt_cur, m)
            for rep in range(body_reps):
                if rep + 1 < body_reps:
                    xnt_next = st["xnt"].tile([P, KT, TPC], MM_DT, name="xnt")

                    def interleave(ci, _xn=xnt_next):
                        for m in _A_SCHED.get(ci, ()):
                            _phase_a_tile(tc, st, x, _xn, m)
                else:
                    xnt_next = None

                    def interleave(ci):
                        pass
                _phase_b(tc, st, wts, outs, xnt_cur, interleave)
                xnt_cur = xnt_next
    nc.compile()
    return nc


# phase-A m-tiles of the NEXT rep emitted after these phase-B chunk indices
_A_SCHED = {0: (0, 1), 1: (2, 3), 2: (4, 5), 3: (6, 7)}


def _make_state(ctx, tc, cosT, sinT, biases):
    """Pools + one-time constants, shared by all body reps."""
    nc = tc.nc
    st = {}
    singles = ctx.enter_context(tc.tile_pool(name="singles", bufs=1))
    st["xpool"] = ctx.enter_context(tc.tile_pool(name="xpool", bufs=3))
    st["xh"] = ctx.enter_context(tc.tile_pool(name="xh", bufs=3))
    st["stats"] = ctx.enter_context(tc.tile_pool(name="stats", bufs=4))
    st["xnt"] = ctx.enter_context(tc.tile_pool(name="xnt", bufs=2))
    st["wt"] = ctx.enter_context(tc.tile_pool(name="wt", bufs=2))
    st["stage"] = ctx.enter_context(tc.tile_pool(name="stage", bufs=4))
    st["rope"] = ctx.enter_context(tc.tile_pool(name="rope", bufs=3))
    # All 8 PSUM banks rotate through matmul accumulators
    st["psumB"] = ctx.enter_context(
        tc.tile_pool(name="psumB", bufs=8, space="PSUM"))

    eps_t = singles.tile([P, 1], F32)
    nc.vector.memset(eps_t, LN_EPS)
    st["eps"] = eps_t
    cos_sb = singles.tile([P, MT, HD // 2], F32)
    nc.sync.dma_start(out=cos_sb, in_=cosT)
    st["cos"] = cos_sb
    sin_sb = singles.tile([P, MT, HD // 2], F32)
    nc.sync.dma_start(out=sin_sb, in_=sinT)
    st["sin"] = sin_sb
    st["bias"] = None
    if biases is not None:
        st["bias"] = []
        for bi, b_dram in enumerate(biases):
            b_sb = singles.tile([P, DM], OUT_DT, name=f"bias{bi}")
            # broadcast the [1, DM] row into all 128 partitions
            b_bc = bass.AP(tensor=b_dram.tensor, offset=b_dram.offset,
                           ap=[[0, P], b_dram.ap[1]])
            nc.sync.dma_start(out=b_sb, in_=b_bc)
            st["bias"].append(b_sb)
    return st


def _phase_a_tile(tc, st, x, xnt, m):
    """LayerNorm of one 128-token m-tile, then DMA-xbar transpose into xnt."""
    nc = tc.nc
    x_t = st["xpool"].tile([P, DM], F32)
    nc.sync.dma_start(out=x_t, in_=x[m * P:(m + 1) * P, :])

    xg = x_t.rearrange("p (g s) -> p g s", s=512)
    stt = st["stats"].tile([P, 4, nc.vector.BN_STATS_DIM], F32)
    for g in range(4):
        nc.vector.bn_stats(out=stt[:, g, :], in_=xg[:, g, :])
    mv = st["stats"].tile([P, nc.vector.BN_AGGR_DIM], F32)
    nc.vector.bn_aggr(out=mv, in_=stt)

    # rsig = 1/sqrt(var + eps)
    rsig = st["stats"].tile([P, 1], F32)
    nc.scalar.activation(out=rsig, in_=mv[:, 1:2],
                         func=mybir.ActivationFunctionType.Sqrt,
                         bias=st["eps"], scale=1.0)
    nc.vector.reciprocal(out=rsig, in_=rsig)

    # xh = (x - mu) * rsig, cast to the matmul dtype
    xh = st["xh"].tile([P, DM], MM_DT)
    nc.vector.tensor_scalar(out=xh, in0=x_t,
                            scalar1=mv[:, 0:1], scalar2=rsig,
                            op0=mybir.AluOpType.subtract,
                            op1=mybir.AluOpType.mult)

    # One xbar transpose per m-tile: xnt[p, k, t] = xh[t, k*128+p]
    nc.scalar.dma_start_transpose(out=xnt[:, :, m * P:(m + 1) * P], in_=xh)


def _phase_b(tc, st, wts, outs, xnt, interleave):
    nc = tc.nc
    cos_sb, sin_sb = st["cos"], st["sin"]

    # ---- Phase B: QKV matmuls + RoPE + store ----
    # Weight chunks are prefetched at depth 2: chunk i+2's DMA is emitted
    # right after chunk i's matmuls, giving the transfer a full chunk of
    # compute (~50us) to land before it is consumed.
    chunks = [(wi, c) for wi in range(3) for c in range(NCH)]

    def fetch_w(i):
        wi, c = chunks[i]
        w_sb = st["wt"].tile([P, KT * ECW], MM_DT)
        nc.sync.dma_start(out=w_sb, in_=wts[wi][c])
        return w_sb

    wbufs = {0: fetch_w(0), 1: fetch_w(1)}

    for ci, (wi, c) in enumerate(chunks):
        w_sb = wbufs.pop(ci)
        o_dram = outs[wi]
        for m in range(MT):
            accs = [st["psumB"].tile([P, 512], F32, space="PSUM",
                                     name="psB")
                    for _ in range(NB)]
            for n in range(NB):
                for k in range(KT):
                    off = k * ECW + n * 512
                    nc.tensor.matmul(
                        accs[n], lhsT=xnt[:, k, m * P:(m + 1) * P],
                        rhs=w_sb[:, off:off + 512],
                        start=(k == 0), stop=(k == KT - 1),
                    )

            stg = st["stage"].tile([P, ECW], OUT_DT)
            for n in range(NB):
                nc.scalar.activation(
                    out=stg[:, n * 512:(n + 1) * 512], in_=accs[n],
                    func=mybir.ActivationFunctionType.Copy)

            if st["bias"] is not None:  # folded ln_beta (zero for this net)
                nc.vector.tensor_add(
                    stg, stg, st["bias"][wi][:, c * ECW:(c + 1) * ECW])

            if wi < 2:  # rope on q and k
                # q: all 8 heads on VectorE. k: heads split DVE/GpSimd so
                # the per-group RoPE latency stays well under the matmul
                # group time (GpSimd is ~1.9x slower per element).
                if wi == 0:
                    splits = [(nc.vector, 0, HPC)]
                else:
                    splits = [(nc.vector, 0, HPC // 2),
                              (nc.gpsimd, HPC // 2, HPC)]
                ov = stg.rearrange("p (h d) -> p h d", d=HD)
                cos_m = cos_sb[:, m, :]
                sin_m = sin_sb[:, m, :]
                for eng, h0, h1 in splits:
                    nh = h1 - h0
                    q1 = ov[:, h0:h1, 0:HD // 2]
                    q2 = ov[:, h0:h1, HD // 2:HD]
                    cos_b = bass.AP(tensor=cos_m.tensor, offset=cos_m.offset,
                                    ap=[cos_m.ap[0], [0, nh], cos_m.ap[1]])
                    sin_b = bass.AP(tensor=sin_m.tensor, offset=sin_m.offset,
                                    ap=[sin_m.ap[0], [0, nh], sin_m.ap[1]])
                    ta = st["rope"].tile([P, nh, HD // 2], OUT_DT,
                                         name=f"ropeA{wi}{h0}")
                    tb = st["rope"].tile([P, nh, HD // 2], OUT_DT,
                                         name=f"ropeB{wi}{h0}")
                    eng.tensor_mul(ta, q1, sin_b)      # A = q1*sin
                    eng.tensor_mul(tb, q2, sin_b)      # B = q2*sin
                    eng.tensor_mul(q1, q1, cos_b)      # q1 = q1*cos
                    eng.tensor_sub(q1, q1, tb)         # q1 -= B
                    eng.tensor_mul(q2, q2, cos_b)      # q2 = q2*cos
                    eng.tensor_add(q2, q2, ta)         # q2 += A

            # v (no RoPE) stores from the ACT queue: zero-wait issue right
            # after ACT's own PSUM->SBUF copy, and it keeps RoPE-gated
            # stores off the SP queue ahead of weight prefetches.
            dma_eng = nc.scalar if wi == 2 else nc.sync
            dma_eng.dma_start(
                out=o_dram[m * P:(m + 1) * P, c * ECW:(c + 1) * ECW],
                in_=stg)

        if ci + 2 < len(chunks):
            wbufs[ci + 2] = fetch_w(ci + 2)
        interleave(ci)


def _host_prep(x, ln_gamma, ln_beta, wq, wk, wv):
    """Shard/layout inputs. Returns per-core input maps.

    ln_gamma is folded into the projection weights (q = xn_hat @ (gamma*W).T);
    ln_beta folds to a per-output-column bias beta @ W.T, included in the
    maps only when nonzero (the kernel program is built accordingly).
    """
    xf = np.ascontiguousarray(x.reshape(TOK, DM), dtype=np.float32)
    wdt = mybir.dt.np(MM_DT)
    gamma = np.asarray(ln_gamma, np.float32)
    beta = np.asarray(ln_beta, np.float32)

    def tile_w(w):
        wt = np.asarray(w, np.float32).T * gamma[:, None]  # [d, e]
        # [NCH, P, KT*ECW]: [c, p, k*ECW+j] = wt[k*128+p, c*ECW+j]
        t = wt.reshape(KT, P, NCH, ECW).transpose(2, 1, 0, 3)
        return np.ascontiguousarray(t).reshape(NCH, P, KT * ECW).astype(wdt)

    wq_t, wk_t, wv_t = tile_w(wq), tile_w(wk), tile_w(wv)
    with_bias = bool(np.any(beta))
    if with_bias:
        odt = mybir.dt.np(OUT_DT)
        b_maps = {
            f"b{n}": np.ascontiguousarray(
                (beta @ np.asarray(w, np.float32).T).reshape(1, DM)
                .astype(odt))
            for n, w in (("q", wq), ("k", wk), ("v", wv))
        }
    else:
        b_maps = {}

    # Build RoPE tables with jax.numpy, matching the reference's fp32 trig
    # bit-for-bit (numpy's fp32 cos differs by ~3e-4 at large arguments).
    import jax.numpy as jnp
    inv_freq = 1.0 / (ROPE_BASE ** (jnp.arange(0, HD, 2, dtype=jnp.float32) / HD))
    t = jnp.arange(N, dtype=jnp.float32)
    freqs = jnp.einsum("i,j->ij", t, inv_freq)  # [N, 64]
    cos_full = np.asarray(jnp.cos(freqs), dtype=np.float32)
    sin_full = np.asarray(jnp.sin(freqs), dtype=np.float32)

    in_maps = []
    for c in range(NCORES):
        pos0 = (c * TPC) % N
        cos_c = np.ascontiguousarray(
            cos_full[pos0:pos0 + TPC].reshape(MT, P, HD // 2).transpose(1, 0, 2))
        sin_c = np.ascontiguousarray(
            sin_full[pos0:pos0 + TPC].reshape(MT, P, HD // 2).transpose(1, 0, 2))
        in_maps.append({
            "x": np.ascontiguousarray(xf[c * TPC:(c + 1) * TPC]),
            "wq": wq_t, "wk": wk_t, "wv": wv_t,
            "cosT": cos_c, "sinT": sin_c,
            **b_maps,
        })
    return in_maps


def _assemble(res_list, name):
    full = np.concatenate([res_list[c][name] for c in range(NCORES)], axis=0)
    return np.ascontiguousarray(
        full.reshape(B, N, HEADS, HD).transpose(0, 2, 1, 3)
        .astype(np.float32))


def kernel(x, ln_gamma, ln_beta, wq, wk, wv, num_heads, _trace=False):
    assert int(num_heads) == HEADS
    in_maps = _host_prep(x, ln_gamma, ln_beta, wq, wk, wv)
    with_bias = "bq" in in_maps[0]
    key = f"nc_bias{with_bias}"
    if key not in _CACHE:
        _CACHE[key] = _build_nc(with_bias=with_bias)
    nc = _CACHE[key]
    r = run_bass_kernel_spmd(nc, in_maps, core_ids=list(range(NCORES)),
                             trace=_trace)
    if _trace:
        _CACHE["last_results"] = r
    q = _assemble(r.results, "q_out")
    k = _assemble(r.results, "k_out")
    v = _assemble(r.results, "v_out")
    return q, k, v


# revision 36
# speedup vs baseline: 1.4147x; 1.0087x over previous
"""Trainium2 Bass kernel: LayerNorm + QKV projection + RoPE (dense transformer).

Full inputs in, full outputs out. Internally shards the 8192 token rows
(b=2 x n=4096) across 8 NeuronCores (data parallel, 1024 tokens/core).

Per-core pipeline (v6 — PE is a pure fp16 matmul stream):
  Phase A: DMA x tile [128, 2048]; LayerNorm via bn_stats/bn_aggr (VectorE)
     writing normalized fp16; one DMA-xbar transpose per m-tile lands it in
     xnt [d, k, t]. ln_gamma is folded into the weights host-side (and
     ln_beta, when nonzero, becomes a per-column bias added post-matmul), so
     no PE transposes or ScalarE fixup copies exist at all. xnt is
     double-buffered; the next rep's phase A is emitted between this rep's
     phase-B chunks (software pipeline), so the PE never idles.
  Phase B: QKV matmuls in fp16, 512-col moving operands accumulated over 16
     k-tiles in PSUM (8-bank rotation); weight half-chunks stream via one
     coarse DMA each (32KB/partition contiguous), prefetched 2 chunks deep;
     ScalarE drains PSUM->SBUF casting fp16; RoPE on VectorE (q) and
     VectorE+GpSimd (k); fp16 output blocks DMA out (v from the ACT queue).

Pools and constants are created once; body repetitions (used by the timing
harness) share them, so reps pipeline back-to-back without drain barriers.
"""

import os
from contextlib import ExitStack

import numpy as np

import concourse.bass as bass
import concourse.tile as tile
from concourse import bacc, mybir
from concourse.bass_utils import run_bass_kernel_spmd

# Problem shapes (hardcoded per contract)
B, N, DM = 2, 4096, 2048
NCORES = 8
TOK = B * N            # 8192 total token rows
TPC = TOK // NCORES    # 1024 tokens per core
P = 128
MT = TPC // P          # 8 m-tiles per core
KT = DM // P           # 16 k-tiles (contraction)
HEADS, HD = 16, 128
ECW = 1024             # weight-chunk width (half the e range)
NCH = DM // ECW        # 2 chunks
NB = ECW // 512        # matmul n-slices per chunk (PSUM bank = 512 fp32)
HPC = ECW // HD        # heads per chunk = 8
LN_EPS = 1e-5
ROPE_BASE = 10000.0

F32 = mybir.dt.float32
# Matmul input dtype: float16 (default; ~tf32 accuracy) or bfloat16.
MM_DT = getattr(mybir.dt, os.environ.get("QKV_MM_DT", "float16"))
# Staging/output dtype: fp16 halves output DMA traffic and doubles RoPE
# vector throughput; quantization (~5e-4 rel) is far inside the 2e-2 gate.
OUT_DT = getattr(mybir.dt, os.environ.get("QKV_OUT_DT", "float16"))

_CACHE = {}


def _build_nc(body_reps=None, with_bias=False):
    if body_reps is None:
        body_reps = int(os.environ.get("QKV_BODY_REPS", "1"))
    nc = bacc.Bacc("TRN2", target_bir_lowering=False, debug=False,
                   enable_asserts=False, num_devices=NCORES)

    x = nc.dram_tensor("x", [TPC, DM], MM_DT, kind="ExternalInput").ap()
    wts = [
        nc.dram_tensor(f"w{n}", [NCH, P, KT * ECW], MM_DT,
                       kind="ExternalInput").ap()
        for n in "qkv"
    ]
    # ln_beta folded through the projections: per-output-column bias,
    # only materialized when ln_beta is nonzero.
    biases = None
    if with_bias:
        biases = [
            nc.dram_tensor(f"b{n}", [1, DM], OUT_DT,
                           kind="ExternalInput").ap()
            for n in "qkv"
        ]
    cosT = nc.dram_tensor("cosT", [P, MT, HD // 2], F32, kind="ExternalInput").ap()
    sinT = nc.dram_tensor("sinT", [P, MT, HD // 2], F32, kind="ExternalInput").ap()
    outs = [
        nc.dram_tensor(f"{n}_out", [TPC, DM], OUT_DT,
                       kind="ExternalOutput").ap()
        for n in "qkv"
    ]

    with tile.TileContext(nc) as tc:
        with ExitStack() as ctx:
            st = _make_state(ctx, tc, cosT, sinT, biases)
            # Software pipeline across body reps: phase A of rep r+1 is
            # emitted tile-by-tile between phase B chunks of rep r, so the
            # PE never sees a long transpose-only (non-matmul) stretch.
            xnt_cur = st["xnt"].tile([P, KT, TPC], MM_DT, name="xnt")
            for m in range(MT):
                _phase_a_tile(tc, st, x, xnt_cur, m)
            for rep in range(body_reps):
                if rep + 1 < body_reps:
                    xnt_next = st["xnt"].tile([P, KT, TPC], MM_DT, name="xnt")

                    def interleave(ci, _xn=xnt_next):
                        for m in _A_SCHED.get(ci, ()):
                            _phase_a_tile(tc, st, x, _xn, m)
                else:
                    xnt_next = None

                    def interleave(ci):
                        pass
                _phase_b(tc, st, wts, outs, xnt_cur, interleave)
                xnt_cur = xnt_next
    nc.compile()
    return nc


# phase-A m-tiles of the NEXT rep emitted after these phase-B chunk indices
_A_SCHED = {0: (0, 1), 1: (2, 3), 2: (4, 5), 3: (6, 7)}


def _make_state(ctx, tc, cosT, sinT, biases):
    """Pools + one-time constants, shared by all body reps."""
    nc = tc.nc
    st = {}
    singles = ctx.enter_context(tc.tile_pool(name="singles", bufs=1))
    st["xpool"] = ctx.enter_context(tc.tile_pool(name="xpool", bufs=3))
    st["xh"] = ctx.enter_context(tc.tile_pool(name="xh", bufs=3))
    st["stats"] = ctx.enter_context(tc.tile_pool(name="stats", bufs=4))
    st["xnt"] = ctx.enter_context(tc.tile_pool(name="xnt", bufs=2))
    st["wt"] = ctx.enter_context(tc.tile_pool(name="wt", bufs=3))
    st["stage"] = ctx.enter_context(tc.tile_pool(name="stage", bufs=4))
    st["rope"] = ctx.enter_context(tc.tile_pool(name="rope", bufs=2))
    # All 8 PSUM banks rotate through matmul accumulators
    st["psumB"] = ctx.enter_context(
        tc.tile_pool(name="psumB", bufs=8, space="PSUM"))

    eps_t = singles.tile([P, 1], F32)
    nc.vector.memset(eps_t, LN_EPS)
    st["eps"] = eps_t
    cos_sb = singles.tile([P, MT, HD // 2], F32)
    nc.sync.dma_start(out=cos_sb, in_=cosT)
    st["cos"] = cos_sb
    sin_sb = singles.tile([P, MT, HD // 2], F32)
    nc.sync.dma_start(out=sin_sb, in_=sinT)
    st["sin"] = sin_sb
    st["bias"] = None
    if biases is not None:
        st["bias"] = []
        for bi, b_dram in enumerate(biases):
            b_sb = singles.tile([P, DM], OUT_DT, name=f"bias{bi}")
            # broadcast the [1, DM] row into all 128 partitions
            b_bc = bass.AP(tensor=b_dram.tensor, offset=b_dram.offset,
                           ap=[[0, P], b_dram.ap[1]])
            nc.sync.dma_start(out=b_sb, in_=b_bc)
            st["bias"].append(b_sb)
    return st


def _phase_a_tile(tc, st, x, xnt, m):
    """LayerNorm of one 128-token m-tile, then DMA-xbar transpose into xnt."""
    nc = tc.nc
    x_t = st["xpool"].tile([P, DM], MM_DT)
    nc.sync.dma_start(out=x_t, in_=x[m * P:(m + 1) * P, :])

    xg = x_t.rearrange("p (g s) -> p g s", s=512)
    stt = st["stats"].tile([P, 4, nc.vector.BN_STATS_DIM], F32)
    for g in range(4):
        nc.vector.bn_stats(out=stt[:, g, :], in_=xg[:, g, :])
    mv = st["stats"].tile([P, nc.vector.BN_AGGR_DIM], F32)
    nc.vector.bn_aggr(out=mv, in_=stt)

    # rsig = 1/sqrt(var + eps)
    rsig = st["stats"].tile([P, 1], F32)
    nc.scalar.activation(out=rsig, in_=mv[:, 1:2],
                         func=mybir.ActivationFunctionType.Sqrt,
                         bias=st["eps"], scale=1.0)
    nc.vector.reciprocal(out=rsig, in_=rsig)

    # xh = (x - mu) * rsig, cast to the matmul dtype
    xh = st["xh"].tile([P, DM], MM_DT)
    nc.vector.tensor_scalar(out=xh, in0=x_t,
                            scalar1=mv[:, 0:1], scalar2=rsig,
                            op0=mybir.AluOpType.subtract,
                            op1=mybir.AluOpType.mult)

    # One xbar transpose per m-tile: xnt[p, k, t] = xh[t, k*128+p]
    nc.scalar.dma_start_transpose(out=xnt[:, :, m * P:(m + 1) * P], in_=xh)


def _phase_b(tc, st, wts, outs, xnt, interleave):
    nc = tc.nc
    cos_sb, sin_sb = st["cos"], st["sin"]

    # ---- Phase B: QKV matmuls + RoPE + store ----
    # Weight chunks are prefetched at depth 2: chunk i+2's DMA is emitted
    # right after chunk i's matmuls, giving the transfer a full chunk of
    # compute (~50us) to land before it is consumed.
    chunks = [(wi, c) for wi in range(3) for c in range(NCH)]

    def fetch_w(i):
        wi, c = chunks[i]
        w_sb = st["wt"].tile([P, KT * ECW], MM_DT)
        nc.sync.dma_start(out=w_sb, in_=wts[wi][c])
        return w_sb

    wbufs = {0: fetch_w(0), 1: fetch_w(1)}

    for ci, (wi, c) in enumerate(chunks):
        if ci + 2 < len(chunks):
            wbufs[ci + 2] = fetch_w(ci + 2)
        w_sb = wbufs.pop(ci)
        o_dram = outs[wi]
        for m in range(MT):
            accs = [st["psumB"].tile([P, 512], F32, space="PSUM",
                                     name="psB")
                    for _ in range(NB)]
            for n in range(NB):
                for k in range(KT):
                    off = k * ECW + n * 512
                    nc.tensor.matmul(
                        accs[n], lhsT=xnt[:, k, m * P:(m + 1) * P],
                        rhs=w_sb[:, off:off + 512],
                        start=(k == 0), stop=(k == KT - 1),
                    )

            stg = st["stage"].tile([P, ECW], OUT_DT)
            for n in range(NB):
                nc.scalar.activation(
                    out=stg[:, n * 512:(n + 1) * 512], in_=accs[n],
                    func=mybir.ActivationFunctionType.Copy)

            if st["bias"] is not None:  # folded ln_beta (zero for this net)
                nc.vector.tensor_add(
                    stg, stg, st["bias"][wi][:, c * ECW:(c + 1) * ECW])

            if wi < 2:  # rope on q and k
                # q: all 8 heads on VectorE. k: heads split DVE/GpSimd so
                # the per-group RoPE latency stays well under the matmul
                # group time (GpSimd is ~1.9x slower per element).
                if wi == 0:
                    splits = [(nc.vector, 0, HPC)]
                else:
                    splits = [(nc.vector, 0, HPC // 2),
                              (nc.gpsimd, HPC // 2, HPC)]
                ov = stg.rearrange("p (h d) -> p h d", d=HD)
                cos_m = cos_sb[:, m, :]
                sin_m = sin_sb[:, m, :]
                for eng, h0, h1 in splits:
                    nh = h1 - h0
                    q1 = ov[:, h0:h1, 0:HD // 2]
                    q2 = ov[:, h0:h1, HD // 2:HD]
                    cos_b = bass.AP(tensor=cos_m.tensor, offset=cos_m.offset,
                                    ap=[cos_m.ap[0], [0, nh], cos_m.ap[1]])
                    sin_b = bass.AP(tensor=sin_m.tensor, offset=sin_m.offset,
                                    ap=[sin_m.ap[0], [0, nh], sin_m.ap[1]])
                    ta = st["rope"].tile([P, nh, HD // 2], OUT_DT,
                                         name=f"ropeA{wi}{h0}")
                    tb = st["rope"].tile([P, nh, HD // 2], OUT_DT,
                                         name=f"ropeB{wi}{h0}")
                    eng.tensor_mul(ta, q1, sin_b)      # A = q1*sin
                    eng.tensor_mul(tb, q2, sin_b)      # B = q2*sin
                    eng.tensor_mul(q1, q1, cos_b)      # q1 = q1*cos
                    eng.tensor_sub(q1, q1, tb)         # q1 -= B
                    eng.tensor_mul(q2, q2, cos_b)      # q2 = q2*cos
                    eng.tensor_add(q2, q2, ta)         # q2 += A

            # v (no RoPE) stores from the ACT queue: zero-wait issue right
            # after ACT's own PSUM->SBUF copy, and it keeps RoPE-gated
            # stores off the SP queue ahead of weight prefetches.
            dma_eng = nc.scalar if wi == 2 else nc.sync
            dma_eng.dma_start(
                out=o_dram[m * P:(m + 1) * P, c * ECW:(c + 1) * ECW],
                in_=stg)

        interleave(ci)


def _host_prep(x, ln_gamma, ln_beta, wq, wk, wv):
    """Shard/layout inputs. Returns per-core input maps.

    ln_gamma is folded into the projection weights (q = xn_hat @ (gamma*W).T);
    ln_beta folds to a per-output-column bias beta @ W.T, included in the
    maps only when nonzero (the kernel program is built accordingly).
    """
    xf = np.ascontiguousarray(x.reshape(TOK, DM), dtype=np.float32)
    wdt = mybir.dt.np(MM_DT)
    gamma = np.asarray(ln_gamma, np.float32)
    beta = np.asarray(ln_beta, np.float32)

    def tile_w(w):
        wt = np.asarray(w, np.float32).T * gamma[:, None]  # [d, e]
        # [NCH, P, KT*ECW]: [c, p, k*ECW+j] = wt[k*128+p, c*ECW+j]
        t = wt.reshape(KT, P, NCH, ECW).transpose(2, 1, 0, 3)
        return np.ascontiguousarray(t).reshape(NCH, P, KT * ECW).astype(wdt)

    wq_t, wk_t, wv_t = tile_w(wq), tile_w(wk), tile_w(wv)
    with_bias = bool(np.any(beta))
    if with_bias:
        odt = mybir.dt.np(OUT_DT)
        b_maps = {
            f"b{n}": np.ascontiguousarray(
                (beta @ np.asarray(w, np.float32).T).reshape(1, DM)
                .astype(odt))
            for n, w in (("q", wq), ("k", wk), ("v", wv))
        }
    else:
        b_maps = {}

    # Build RoPE tables with jax.numpy, matching the reference's fp32 trig
    # bit-for-bit (numpy's fp32 cos differs by ~3e-4 at large arguments).
    import jax.numpy as jnp
    inv_freq = 1.0 / (ROPE_BASE ** (jnp.arange(0, HD, 2, dtype=jnp.float32) / HD))
    t = jnp.arange(N, dtype=jnp.float32)
    freqs = jnp.einsum("i,j->ij", t, inv_freq)  # [N, 64]
    cos_full = np.asarray(jnp.cos(freqs), dtype=np.float32)
    sin_full = np.asarray(jnp.sin(freqs), dtype=np.float32)

    in_maps = []
    for c in range(NCORES):
        pos0 = (c * TPC) % N
        cos_c = np.ascontiguousarray(
            cos_full[pos0:pos0 + TPC].reshape(MT, P, HD // 2).transpose(1, 0, 2))
        sin_c = np.ascontiguousarray(
            sin_full[pos0:pos0 + TPC].reshape(MT, P, HD // 2).transpose(1, 0, 2))
        in_maps.append({
            "x": np.ascontiguousarray(
                xf[c * TPC:(c + 1) * TPC].astype(wdt)),
            "wq": wq_t, "wk": wk_t, "wv": wv_t,
            "cosT": cos_c, "sinT": sin_c,
            **b_maps,
        })
    return in_maps


def _assemble(res_list, name):
    full = np.concatenate([res_list[c][name] for c in range(NCORES)], axis=0)
    return np.ascontiguousarray(
        full.reshape(B, N, HEADS, HD).transpose(0, 2, 1, 3)
        .astype(np.float32))


def kernel(x, ln_gamma, ln_beta, wq, wk, wv, num_heads, _trace=False):
    assert int(num_heads) == HEADS
    in_maps = _host_prep(x, ln_gamma, ln_beta, wq, wk, wv)
    with_bias = "bq" in in_maps[0]
    key = f"nc_bias{with_bias}"
    if key not in _CACHE:
        _CACHE[key] = _build_nc(with_bias=with_bias)
    nc = _CACHE[key]
    r = run_bass_kernel_spmd(nc, in_maps, core_ids=list(range(NCORES)),
                             trace=_trace)
    if _trace:
        _CACHE["last_results"] = r
    q = _assemble(r.results, "q_out")
    k = _assemble(r.results, "k_out")
    v = _assemble(r.results, "v_out")
    return q, k, v
